# revision 9
# baseline (speedup 1.0000x reference)
"""Trainium2 Bass kernel for nn_DecNP (two-stage KNN feature propagation).

Per stage: rank candidates by negE = 2*q.c - |c|^2 as bf16 matmuls with a
21-row hi/mid/lo split of both operands (score err ~8e-6: top-8 selection
matches fp32 on all but ~10 near-tie rows), evacuate PSUM->SBUF with one
ACT copy per 1024-col chunk, top-8 via DVE max8/find_index8.

Gathers are split so they pipeline with the scan phase: the 336-byte META
rows (xyz | perc | normalized dirs) come from host-packed input tables and
are gathered DURING the ranking/scan phase, with the direction-mask weight
math interleaved on the Vector engine; the 1536-byte FEATURE rows are
gathered in a second phase -- for stage 1 directly from the AllGather
output buffer, so no on-device table assembly is needed.  Interpolation is
PE diagonal-weight matmuls; the skip-add + L2 normalize runs per-tile in
the same pipeline once the scalar AllReduce lands.

Sharding: query rows split across 8 cores.  Stage-0 output is AllGather'd
(it is stage-1's feature table); the scalar mean of de_k_weight_sum is
AllReduce'd per stage.
"""
import sys

for _p in ("/opt/trn_rl_repo", "/root/.axon_site/_ro/trn_rl_repo", "/root/.axon_site"):
    if _p not in sys.path:
        sys.path.append(_p)

import ml_dtypes
import numpy as np

import concourse.bacc as bacc
import concourse.bass as bass
import concourse.bass_isa as bass_isa
import concourse.mybir as mybir
from concourse.masks import make_identity
from concourse.tile import TileContext

NCORES = 8
P = 128
D = 768
K = 8
M = 20
KR = 21        # rows in the hi/mid/lo split score matmul
GAMMA = 0.85
EPS_DIR = 1e-8
METAB = 168    # bf16 columns holding the 84 fp32 meta words (bitcast)
CH = 1024      # rank matmul chunk (two 512-col PSUM-bank matmuls)
BF16 = mybir.dt.bfloat16
F32 = mybir.dt.float32
U32 = mybir.dt.uint32
X = mybir.AxisListType.X
Copy = mybir.ActivationFunctionType.Copy
Sqrt = mybir.ActivationFunctionType.Sqrt
Square = mybir.ActivationFunctionType.Square
Abs = mybir.ActivationFunctionType.Abs
Mult = mybir.AluOpType.mult
Add = mybir.AluOpType.add

ST0 = dict(S=1024, Q=512, NT=4096)
ST1 = dict(S=4096, Q=2048, NT=16384)
C_SCAL = 0.3  # N == 4*S in both stages

RG = [list(range(NCORES))]

_CACHE = {}


class Stage:
    def __init__(self, nc, pools, *, st, S, Q, NT, tmeta, tfeat, qst, cst,
                 qxp, p1, out_rows, sum_in, sum_out, out_bf):
        self.__dict__.update(locals())
        self.n_qt = Q // P
        self.tma = tmeta.ap()
        self.tfa = tfeat.ap()
        self.p1a = p1.ap()
        self.ora = out_rows.ap()

    def emit_tables(self):
        nc, pools = self.nc, self.pools
        st, S, Q = self.st, self.S, self.Q
        self.qsT = pools["tbl"].tile([KR, Q], BF16, tag=f"qsT_{st}")
        nc.sync.dma_start(out=self.qsT[:, :], in_=self.qst.ap()[:, :])
        self.csT = pools["tbl"].tile([KR, S], BF16, tag=f"csT_{st}")
        csa = self.cst.ap()
        for c in range(0, S, CH):  # chunked so the first matmul starts early
            nc.sync.dma_start(out=self.csT[:, c:c + CH], in_=csa[:, c:c + CH])
        self.qxall = pools["tbl"].tile([P, self.n_qt, 3], F32, tag=f"qx_{st}")
        nc.sync.dma_start(out=self.qxall[:, :, :], in_=self.qxp.ap()[:, :, :])
        self.idxall = pools["tbl"].tile([P, self.n_qt, K], U32, tag=f"idx_{st}")
        self.bestall = pools["tbl"].tile([P, self.n_qt, K], F32, tag=f"best_{st}")
        self.wpall = pools["tbl"].tile([P, self.n_qt, K], BF16, tag=f"wp_{st}")
        self.acc = pools["tbl"].tile([P, 1], F32, tag=f"acc_{st}")
        nc.vector.memset(self.acc[:, :], 0.0)
        self.f1keep = pools["keep"].tile([P, self.n_qt, D], BF16, tag=f"f1k_{st}")

    def emit_part_a(self, t):
        """bf16 ranking matmuls + top-8 values/indices for tile t."""
        nc, pools = self.nc, self.pools
        S = self.S
        rs = slice(t * P, (t + 1) * P)
        negE = pools["neg"].tile([P, S], F32, tag="negE")
        for c in range(0, S, CH):
            pe = pools["pe"].tile([P, CH], F32, tag="pe")
            for h in range(CH // 512):
                nc.tensor.matmul(out=pe[:, h * 512:(h + 1) * 512],
                                 lhsT=self.qsT[:, rs],
                                 rhs=self.csT[:, c + h * 512:c + (h + 1) * 512],
                                 start=True, stop=True)
            nc.scalar.activation(out=negE[:, c:c + CH], in_=pe[:, :], func=Copy)
        nc.vector.max(out=self.bestall[:, t, :], in_=negE[:, :])
        nc.vector.max_index(out=self.idxall[:, t, :],
                            in_max=self.bestall[:, t, :], in_values=negE[:, :])

    def emit_gather_meta(self, t):
        nc, pools = self.nc, self.pools
        gtm = pools["gtm"].tile([P, K, METAB], BF16, tag="gtm")
        for k in range(K):
            nc.gpsimd.indirect_dma_start(
                out=gtm[:, k, :], out_offset=None, in_=self.tma[:, :],
                in_offset=bass.IndirectOffsetOnAxis(
                    ap=self.idxall[:, t, k:k + 1], axis=0))
        return gtm

    def emit_gather_feat(self, t):
        nc, pools = self.nc, self.pools
        gtf = pools["gtf"].tile([P, K, D], BF16, tag="gtf")
        for k in range(K):
            nc.gpsimd.indirect_dma_start(
                out=gtf[:, k, :], out_offset=None, in_=self.tfa[:, :],
                in_offset=bass.IndirectOffsetOnAxis(
                    ap=self.idxall[:, t, k:k + 1], axis=0))
        return gtf

    def emit_b1(self, t, gtm):
        """Direction-mask weights for tile t from gathered meta rows."""
        nc, pools = self.nc, self.pools
        gmeta = gtm[:, :, :].bitcast(F32)  # [P, K, 84] fp32 view

        vec = pools["work"].tile([P, K, 3], F32, tag="vec")
        nc.vector.tensor_tensor(
            out=vec[:, :, :], in0=gmeta[:, :, 0:3],
            in1=self.qxall[:, t, :].unsqueeze(1).to_broadcast([P, K, 3]),
            op=mybir.AluOpType.subtract)
        v2 = pools["work"].tile([P, K, 3], F32, tag="v2")
        nc.vector.tensor_mul(v2[:, :, :], vec[:, :, :], vec[:, :, :])
        d2 = pools["work"].tile([P, K], F32, tag="d2")
        nc.vector.reduce_sum(out=d2[:, :], in_=v2[:, :, :], axis=X)
        dist = pools["work"].tile([P, K], F32, tag="dist")
        nc.scalar.activation(out=dist[:, :], in_=d2[:, :], func=Sqrt)
        nc.vector.tensor_scalar_add(dist[:, :], dist[:, :], EPS_DIR)
        riv = pools["work"].tile([P, K], F32, tag="riv")
        nc.vector.reciprocal(riv[:, :], dist[:, :])
        vecn = pools["work"].tile([P, K, 3], F32, tag="vecn")
        nc.vector.tensor_mul(vecn[:, :, :], vec[:, :, :],
                             riv[:, :].unsqueeze(2).to_broadcast([P, K, 3]))

        prod = pools["work"].tile([P, K, M, 3], F32, tag="prod")
        nc.vector.tensor_mul(
            prod[:, :, :, :],
            gmeta[:, :, 23:83].rearrange("p k (m c) -> p k m c", c=3),
            vecn[:, :, :].unsqueeze(2).to_broadcast([P, K, M, 3]),
        )
        simm = pools["work"].tile([P, K, M], F32, tag="simm")
        nc.vector.reduce_sum(out=simm[:, :, :], in_=prod[:, :, :, :], axis=X)
        absm = pools["work"].tile([P, K, M], F32, tag="absm")
        nc.scalar.activation(out=absm[:, :, :], in_=simm[:, :, :], func=Abs)
        mask = pools["work"].tile([P, K, M], F32, tag="mask")
        nc.vector.tensor_scalar(out=mask[:, :, :], in0=absm[:, :, :],
                                scalar1=GAMMA, scalar2=None,
                                op0=mybir.AluOpType.is_gt)
        mw = pools["work"].tile([P, K, M], F32, tag="mw")
        nc.vector.tensor_mul(mw[:, :, :], mask[:, :, :], gmeta[:, :, 3:23])
        dkw = pools["work"].tile([P, K], F32, tag="dkw")
        nc.vector.reduce_sum(out=dkw[:, :], in_=mw[:, :, :], axis=X)

        dkws = pools["work"].tile([P, 1], F32, tag="dkws")
        nc.vector.reduce_sum(out=dkws[:, :], in_=dkw[:, :], axis=X)
        nc.vector.tensor_scalar_add(dkws[:, :], dkws[:, :], 1e-8)
        r1 = pools["work"].tile([P, 1], F32, tag="r1")
        nc.vector.reciprocal(r1[:, :], dkws[:, :])
        wn = pools["work"].tile([P, K], F32, tag="wn")
        nc.vector.tensor_scalar(out=wn[:, :], in0=dkw[:, :], scalar1=r1[:, 0:1],
                                scalar2=1e-6, op0=Mult, op1=Add)
        nc.vector.tensor_scalar_add(wn[:, :], wn[:, :], 1e-10)
        nr2 = pools["work"].tile([P, 1], F32, tag="nr2")
        nc.vector.reduce_sum(out=nr2[:, :], in_=wn[:, :], axis=X)
        nc.vector.tensor_scalar_add(nr2[:, :], nr2[:, :], 1e-8)
        r2 = pools["work"].tile([P, 1], F32, tag="r2")
        nc.vector.reciprocal(r2[:, :], nr2[:, :])
        wp = pools["work"].tile([P, K], F32, tag="wp")
        nc.vector.tensor_scalar(out=wp[:, :], in0=wn[:, :], scalar1=r2[:, 0:1],
                                scalar2=dkws[:, 0:1], op0=Mult, op1=Mult)
        nc.vector.tensor_copy(self.wpall[:, t, :], wp[:, :])

        par = pools["work"].tile([P, 1], F32, tag="par")
        nc.gpsimd.partition_all_reduce(par[:, :], dkws[:, :], channels=P,
                                       reduce_op=bass_isa.ReduceOp.add)
        nc.vector.tensor_add(self.acc[:, :], self.acc[:, :], par[:, :])

    def emit_b2(self, t, gtf):
        """Diagonal-weight interpolation matmuls for tile t."""
        nc, pools = self.nc, self.pools
        dW = pools["dw"].tile([P, K, P], BF16, tag="dW")
        nc.vector.tensor_mul(
            dW[:, :, :],
            self.identb[:, :].unsqueeze(1).to_broadcast([P, K, P]),
            self.wpall[:, t, :].unsqueeze(2).to_broadcast([P, K, P]),
        )
        po = pools["po"].tile([P, D], F32, tag="po")
        for k in range(K):
            for c0, c1 in ((0, 512), (512, D)):
                nc.tensor.matmul(out=po[:, c0:c1], lhsT=dW[:, k, :],
                                 rhs=gtf[:, k, c0:c1],
                                 start=(k == 0), stop=(k == K - 1))
        nc.scalar.activation(out=self.f1keep[:, t, :], in_=po[:, :], func=Copy)

    def emit_allreduce_post(self):
        nc = self.nc
        nc.sync.dma_start(out=self.sum_in.ap()[:, :], in_=self.acc[0:1, 0:1])
        nc.gpsimd.collective_compute(
            "AllReduce", mybir.AluOpType.add, replica_groups=RG,
            ins=[self.sum_in.ap()], outs=[self.sum_out.ap()],
        )

    def emit_allreduce_read(self):
        nc, pools = self.nc, self.pools
        sg = pools["tbl"].tile([P, 1], F32, tag=f"sg_{self.st}")
        nc.sync.dma_start(out=sg[0:1, :], in_=self.sum_out.ap()[:, :])
        sgb = pools["tbl"].tile([P, 1], F32, tag=f"sgb_{self.st}")
        nc.gpsimd.partition_broadcast(sgb[:, :], sg[0:1, :], channels=P)
        scal = pools["tbl"].tile([P, 1], F32, tag=f"scal_{self.st}")
        nc.vector.tensor_scalar(out=scal[:, :], in0=sgb[:, :],
                                scalar1=C_SCAL / self.NT, scalar2=1e-8,
                                op0=Mult, op1=Add)
        self.scal = scal

    def emit_def(self, t):
        """normalize(f1 + scal * p1) -> out rows, one tile."""
        nc, pools = self.nc, self.pools
        rs = slice(t * P, (t + 1) * P)
        p1t = pools["f1"].tile([P, D], F32, tag="p1t")
        nc.sync.dma_start(out=p1t[:, :], in_=self.p1a[rs, :])
        o = pools["f1"].tile([P, D], F32, tag="o")
        nc.vector.affine_then_add(o[:, :], p1t[:, :], self.f1keep[:, t, :],
                                  scale=self.scal[:, 0:1], bias=0.0)
        junk = pools["f1"].tile([P, D], F32, tag="junk")
        ss = pools["work"].tile([P, 1], F32, tag="ss")
        nc.scalar.activation(out=junk[:, :], in_=o[:, :], func=Square,
                             accum_out=ss[:, :])
        nn = pools["work"].tile([P, 1], F32, tag="nn")
        nc.scalar.activation(out=nn[:, :], in_=ss[:, :], func=Sqrt)
        nc.vector.tensor_scalar_max(nn[:, :], nn[:, :], 1e-12)
        ri = pools["work"].tile([P, 1], F32, tag="ri")
        nc.vector.reciprocal(ri[:, :], nn[:, :])
        res = pools["f1"].tile([P, D], BF16 if self.out_bf else F32, tag="res")
        nc.vector.tensor_scalar(out=res[:, :], in0=o[:, :],
                                scalar1=ri[:, 0:1], scalar2=None, op0=Mult)
        nc.sync.dma_start(out=self.ora[rs, :], in_=res[:, :])


def build():
    if "nc" in _CACHE:
        return _CACHE["nc"]
    nc = bacc.Bacc("TRN2", num_devices=NCORES)

    # host-packed inputs
    tm0 = nc.dram_tensor("tm0", [ST0["S"], METAB], BF16, kind="ExternalInput")
    tf0 = nc.dram_tensor("tf0", [ST0["S"], D], BF16, kind="ExternalInput")
    tm1 = nc.dram_tensor("tm1", [ST1["S"], METAB], BF16, kind="ExternalInput")
    qs0 = nc.dram_tensor("qs0", [KR, ST0["Q"]], BF16, kind="ExternalInput")
    qs1 = nc.dram_tensor("qs1", [KR, ST1["Q"]], BF16, kind="ExternalInput")
    cs0 = nc.dram_tensor("cs0", [KR, ST0["S"]], BF16, kind="ExternalInput")
    cs1 = nc.dram_tensor("cs1", [KR, ST1["S"]], BF16, kind="ExternalInput")
    qx0 = nc.dram_tensor("qx0", [P, ST0["Q"] // P, 3], F32, kind="ExternalInput")
    qx1 = nc.dram_tensor("qx1", [P, ST1["Q"] // P, 3], F32, kind="ExternalInput")
    p10 = nc.dram_tensor("p10", [ST0["Q"], D], F32, kind="ExternalInput")
    p11 = nc.dram_tensor("p11", [ST1["Q"], D], F32, kind="ExternalInput")

    out1 = nc.dram_tensor("out1", [ST1["Q"], D], F32, kind="ExternalOutput")

    p2s = nc.dram_tensor("p2s", [ST0["Q"], D], BF16)
    p2full = nc.dram_tensor("p2full", [ST1["S"], D], BF16, addr_space="Shared")
    # gather sources must be plain internal DRAM tensors
    tm0i = nc.dram_tensor("tm0i", [ST0["S"], METAB], BF16)
    tf0i = nc.dram_tensor("tf0i", [ST0["S"], D], BF16)
    tm1i = nc.dram_tensor("tm1i", [ST1["S"], METAB], BF16)
    tf1 = nc.dram_tensor("tf1", [ST1["S"], D], BF16)
    s0in = nc.dram_tensor("s0in", [1, 1], F32)
    s0out = nc.dram_tensor("s0out", [1, 1], F32, addr_space="Shared")
    s1in = nc.dram_tensor("s1in", [1, 1], F32)
    s1out = nc.dram_tensor("s1out", [1, 1], F32, addr_space="Shared")

    with TileContext(nc) as tc:
        import contextlib
        with contextlib.ExitStack() as ctx:
            pools = {
                "const": ctx.enter_context(tc.tile_pool(name="const", bufs=1)),
                "tbl": ctx.enter_context(tc.tile_pool(name="tbl", bufs=1)),
                "keep": ctx.enter_context(tc.tile_pool(name="keep", bufs=1)),
                "work": ctx.enter_context(tc.tile_pool(name="work", bufs=2)),
                "neg": ctx.enter_context(tc.tile_pool(name="neg", bufs=2)),
                "gtm": ctx.enter_context(tc.tile_pool(name="gtm", bufs=4)),
                "gtf": ctx.enter_context(tc.tile_pool(name="gtf", bufs=4)),
                "dw": ctx.enter_context(tc.tile_pool(name="dw", bufs=2)),
                "f1": ctx.enter_context(tc.tile_pool(name="f1", bufs=2)),
                "pe": ctx.enter_context(tc.tile_pool(name="pe", bufs=2, space="PSUM")),
                "po": ctx.enter_context(tc.tile_pool(name="po", bufs=2, space="PSUM")),
            }
            identf = pools["const"].tile([P, P], F32, tag="identf")
            make_identity(nc, identf[:, :])
            identb = pools["const"].tile([P, P], BF16, tag="identb")
            nc.scalar.activation(out=identb[:, :], in_=identf[:, :], func=Copy)

            nc.sync.dma_start(out=tm0i.ap()[:, :], in_=tm0.ap()[:, :])
            nc.sync.dma_start(out=tf0i.ap()[:, :], in_=tf0.ap()[:, :])
            nc.sync.dma_start(out=tm1i.ap()[:, :], in_=tm1.ap()[:, :])

            s0 = Stage(nc, pools, st=0, S=ST0["S"], Q=ST0["Q"], NT=ST0["NT"],
                       tmeta=tm0i, tfeat=tf0i, qst=qs0, cst=cs0, qxp=qx0,
                       p1=p10, out_rows=p2s, sum_in=s0in, sum_out=s0out,
                       out_bf=True)
            s1 = Stage(nc, pools, st=1, S=ST1["S"], Q=ST1["Q"], NT=ST1["NT"],
                       tmeta=tm1i, tfeat=tf1, qst=qs1, cst=cs1, qxp=qx1,
                       p1=p11, out_rows=out1, sum_in=s1in, sum_out=s1out,
                       out_bf=False)
            s0.identb = identb[:, :]
            s1.identb = identb[:, :]

            s0.emit_tables()
            s1.emit_tables()

            # stage 0 (features are an input, so B1+B2 run together)
            for t in range(s0.n_qt):
                s0.emit_part_a(t)
            gm = {0: s0.emit_gather_meta(0), 1: s0.emit_gather_meta(1)}
            gf = {0: s0.emit_gather_feat(0), 1: s0.emit_gather_feat(1)}
            for t in range(s0.n_qt):
                if t + 2 < s0.n_qt:
                    gm[t + 2] = s0.emit_gather_meta(t + 2)
                    gf[t + 2] = s0.emit_gather_feat(t + 2)
                s0.emit_b1(t, gm.pop(t))
                s0.emit_b2(t, gf.pop(t))
            s0.emit_allreduce_post()

            # stage-1 scan phase with meta gathers + weight math interleaved;
            # the stage-0 tail (AllReduce read, normalize, AllGather, table
            # copy) is spliced in where its inputs are just-ready so no
            # engine queue head-of-line blocks on a collective.
            gm1 = {}
            for t in range(s1.n_qt):
                s1.emit_part_a(t)
                gm1[t] = s1.emit_gather_meta(t)
                if t >= 2:
                    s1.emit_b1(t - 2, gm1.pop(t - 2))
                if t == 10:
                    s0.emit_allreduce_read()
                if t == 12:
                    for td in range(s0.n_qt):
                        s0.emit_def(td)
                    nc.gpsimd.collective_compute(
                        "AllGather", mybir.AluOpType.bypass, replica_groups=RG,
                        ins=[p2s.ap()], outs=[p2full.ap()],
                    )
                    nc.sync.dma_start(out=tf1.ap()[:, :], in_=p2full.ap()[:, :])
            s1.emit_b1(s1.n_qt - 2, gm1.pop(s1.n_qt - 2))
            s1.emit_b1(s1.n_qt - 1, gm1.pop(s1.n_qt - 1))

            # feature gather + interpolation + deferred, one pipeline; the
            # stage-1 AllReduce posts/reads between gather dispatches
            LGD = 5
            gf1 = {}
            for t in range(s1.n_qt):
                gf1[t] = s1.emit_gather_feat(t)
                if t == 4:
                    s1.emit_allreduce_post()
                if t == 6:
                    s1.emit_allreduce_read()
                if t >= 2:
                    s1.emit_b2(t - 2, gf1.pop(t - 2))
                if t >= 2 + LGD:
                    s1.emit_def(t - 2 - LGD)
            for t in range(s1.n_qt - 2, s1.n_qt):
                s1.emit_b2(t, gf1.pop(t))
            for t in range(s1.n_qt - 2 - LGD, s1.n_qt):
                s1.emit_def(t)

    nc.compile()
    _CACHE["nc"] = nc
    return nc


def _bf(x):
    return x.astype(ml_dtypes.bfloat16)


def _bf32(x):
    return x.astype(ml_dtypes.bfloat16).astype(np.float32)


def _score_tables(q, c):
    """21-row hi/mid/lo split operands for negE = 2*q.c - |c|^2 (bf16)."""
    qh = _bf32(q); qm = _bf32(q - qh); ql = _bf32(q - qh - qm)
    ch = _bf32(c); cm = _bf32(c - ch); cl = _bf32(c - ch - cm)
    n2 = -(c * c).sum(-1)
    n2h = _bf32(n2); n2m = _bf32(n2 - n2h); n2l = _bf32(n2 - n2h - n2m)
    ones = np.ones(q.shape[0], np.float32)
    qrows = [2 * qh[:, 0], 2 * qh[:, 1], 2 * qh[:, 2], ones,
             2 * qh[:, 0], 2 * qh[:, 1], 2 * qh[:, 2], ones,
             2 * qm[:, 0], 2 * qm[:, 1], 2 * qm[:, 2], ones,
             2 * qh[:, 0], 2 * qh[:, 1], 2 * qh[:, 2],
             2 * ql[:, 0], 2 * ql[:, 1], 2 * ql[:, 2],
             2 * qm[:, 0], 2 * qm[:, 1], 2 * qm[:, 2]]
    crows = [ch[:, 0], ch[:, 1], ch[:, 2], n2h,
             cm[:, 0], cm[:, 1], cm[:, 2], n2m,
             ch[:, 0], ch[:, 1], ch[:, 2], n2l,
             cl[:, 0], cl[:, 1], cl[:, 2],
             ch[:, 0], ch[:, 1], ch[:, 2],
             cm[:, 0], cm[:, 1], cm[:, 2]]
    return _bf(np.stack(qrows, 0)), _bf(np.stack(crows, 0))


def _meta_block(xyz, perc, dirs):
    """84 fp32 words -> raw bf16 pairs (bit-exact reinterpret)."""
    S = xyz.shape[0]
    meta = np.zeros((S, 84), np.float32)
    meta[:, 0:3] = xyz
    meta[:, 3:23] = perc
    dn = dirs / (np.linalg.norm(dirs, axis=-1, keepdims=True) + EPS_DIR)
    meta[:, 23:83] = dn.reshape(S, 60)
    return meta.view(np.uint16).view(ml_dtypes.bfloat16)  # [S, 168]


def _pack(inputs):
    xyz_c = np.ascontiguousarray(inputs["xyz_c"][0], dtype=np.float32)
    xyz_m = np.ascontiguousarray(inputs["xyz_m"][0], dtype=np.float32)
    xyz_f = np.ascontiguousarray(inputs["xyz_f"][0], dtype=np.float32)
    x_c = np.ascontiguousarray(inputs["x_c"][0], dtype=np.float32)
    x_m = np.ascontiguousarray(inputs["x_m"][0], dtype=np.float32)
    x_f = np.ascontiguousarray(inputs["x_f"][0], dtype=np.float32)
    perc_c = np.ascontiguousarray(inputs["perc_c"][0], dtype=np.float32)
    dir_c = np.ascontiguousarray(inputs["dir_c"][0], dtype=np.float32)
    perc_m = np.ascontiguousarray(inputs["perc_m"][0], dtype=np.float32)
    dir_m = np.ascontiguousarray(inputs["dir_m"][0], dtype=np.float32)

    tm0 = _meta_block(xyz_c, perc_c, dir_c)
    tf0 = _bf(x_c)
    tm1 = _meta_block(xyz_m, perc_m, dir_m)

    cs0q, cs0c = _score_tables(xyz_m, xyz_c)   # full q-side [21, 4096]
    cs1q, cs1c = _score_tables(xyz_f, xyz_m)   # full q-side [21, 16384]

    in_maps = []
    for c in range(NCORES):
        r0 = slice(c * ST0["Q"], (c + 1) * ST0["Q"])
        r1 = slice(c * ST1["Q"], (c + 1) * ST1["Q"])
        in_maps.append({
            "tm0": tm0,
            "tf0": tf0,
            "tm1": tm1,
            "qs0": np.ascontiguousarray(cs0q[:, r0]),
            "qs1": np.ascontiguousarray(cs1q[:, r1]),
            "cs0": cs0c,
            "cs1": cs1c,
            "qx0": np.ascontiguousarray(
                xyz_m[r0].reshape(ST0["Q"] // P, P, 3).transpose(1, 0, 2)),
            "qx1": np.ascontiguousarray(
                xyz_f[r1].reshape(ST1["Q"] // P, P, 3).transpose(1, 0, 2)),
            "p10": np.ascontiguousarray(x_m[r0]),
            "p11": np.ascontiguousarray(x_f[r1]),
        })
    return in_maps


def run_sharded(inputs, trace=False, tmpdir=None):
    """Build + run; returns (full_output, BassKernelResults)."""
    from concourse.bass_utils import run_bass_kernel_spmd
    nc = build()
    in_maps = _pack(inputs)
    res = run_bass_kernel_spmd(nc, in_maps, list(range(NCORES)), trace=trace,
                               tmpdir=tmpdir)
    out = np.concatenate([res.results[c]["out1"] for c in range(NCORES)], axis=0)
    return out.reshape(1, ST1["NT"], D).astype(np.float32), res


def kernel(**inputs) -> np.ndarray:
    out, _ = run_sharded(inputs, trace=False)
    return out
